# revision 5
# baseline (speedup 1.0000x reference)
"""GIN message-passing network (3 GINConv layers + per-graph sum readout) on
8 Trainium2 NeuronCores via Bass/Tile.

Sharding: nodes are partitioned contiguously across the 8 cores (graph
partitioning by destination); edges live on the core owning their dst node.
The padded global node layout is region-major over two halves (A = first half
of every core's slab, B = second half), so the inter-layer exchange is two
independent AllGathers (hfA, hfB), each with a single writer.

Per layer each core aggregates incoming-edge source features with
indirect-DMA row gathers reduced per 128-row dst tile via one-hot matmuls in
PSUM. Region-A contributions for all tiles are computed first and staged in
SBUF; region-B + self follow, which lets the next layer's region-A work (and
this layer's region-A AllGather) overlap the region-B AllGather. The dense
layer runs as z @ W + b with PE transposes feeding fp32r matmuls; ReLU on the
scalar engine. After layer 3 each core computes s = h3 @ Wc per node; the
host does the final per-graph segment sum (bincount over 50k scalars) + bc.
"""
import os
import sys
import math

sys.path.insert(0, "/opt/trn_rl_repo")

import numpy as np

import concourse.bass as bass
import concourse.bacc as bacc
import concourse.mybir as mybir
import concourse.tile as tile
from concourse import bass_utils

NC = 8
P = 128

f32 = mybir.dt.float32
f32r = mybir.dt.float32r
i32 = mybir.dt.int32

PAD_DLOC = 200.0  # one-hot no-match sentinel for padding slots

_program_cache = {}
LAST_EXEC_TIME_NS = None
LAST_RESULTS = None


def _build_schedule(src, dst, N, q, T, n_own):
    """Group edges by (dst core, 128-row dst tile, source region), chunk by 128.

    Global padded ids are region-major over two halves: slab rows [0, ch) of
    every core form region A (ids [0, ch*NC)), rows [ch, n_own) region B.
    Gather indices are stored LOCAL to their region's tensor.

    Returns (K, OA, OB, CA, CB, src_idx_T, dloc_T):
      K[t, r]    chunks for dst-tile t, region r (max over cores)
      OA/OB[t]   column offset of tile t's region-A/B chunks
      CA, CB     total chunks per region (columns: [A chunks | B chunks])
      src_idx_T  [NC, P, CA+CB] int32 region-local source ids (pad: 0)
      dloc_T     [NC, P, CA+CB] f32 dst index within tile (pad: PAD_DLOC)
    """
    E = src.shape[0]
    ch = n_own // 2
    c_e = dst // q
    dl = dst - c_e * q
    t_e = dl // P
    m_e = dl - t_e * P
    sc = src // q
    sr = src - sc * q            # slab row of the source on its owner core
    r_e = sr // ch               # source region 0/1
    srcl = (sr - r_e * ch) + sc * ch   # region-local id: core-major within region

    key = (c_e * T + t_e) * 2 + r_e
    order = np.argsort(key, kind="stable")
    key_s = key[order]
    srcl_s = srcl[order]
    m_s = m_e[order]

    counts = np.bincount(key, minlength=NC * T * 2)
    starts = np.concatenate([[0], np.cumsum(counts)])
    rank = np.arange(E, dtype=np.int64) - starts[key_s]

    K = np.ceil(counts.reshape(NC, T, 2).max(axis=0) / P).astype(np.int64)
    OA = np.concatenate([[0], np.cumsum(K[:, 0])])
    CA = int(OA[-1])
    OB = np.concatenate([[0], np.cumsum(K[:, 1])]) + CA
    CB = int(OB[-1]) - CA

    j = rank // P
    s = rank - j * P
    c_s = key_s // (T * 2)
    t_s = (key_s // 2) % T
    r_s = key_s % 2
    col = np.where(r_s == 0, OA[t_s], OB[t_s]) + j

    C = CA + CB
    src_idx_T = np.zeros((NC, P, C), np.int32)
    dloc_T = np.full((NC, P, C), PAD_DLOC, np.float32)
    src_idx_T[c_s, s, col] = srcl_s.astype(np.int32)
    dloc_T[c_s, s, col] = m_s.astype(np.float32)
    return K, OA, OB, CA, CB, src_idx_T, dloc_T


def _build_program(D, T, K, OA, OB, CA, CB, n_own, N_pad):
    KT = D // P
    C = CA + CB
    ch = n_own // 2
    chg = ch * NC  # rows per region tensor
    T_A = (ch + P - 1) // P  # tiles fully/partly in output chunk A (25 for T=49)

    nc = bacc.Bacc("TRN2", target_bir_lowering=False, debug=False,
                   num_devices=NC)

    xA_in = nc.dram_tensor("xA_in", [chg, D], f32r, kind="ExternalInput").ap()
    xB_in = nc.dram_tensor("xB_in", [chg, D], f32r, kind="ExternalInput").ap()
    x_own = nc.dram_tensor("x_own", [n_own, D], f32r, kind="ExternalInput").ap()
    w_in = [nc.dram_tensor(f"w{l}_in", [D, D], f32r, kind="ExternalInput").ap()
            for l in range(3)]
    b_in = [nc.dram_tensor(f"b{l}_in", [1, D], f32r, kind="ExternalInput").ap()
            for l in range(3)]
    wc_in = nc.dram_tensor("wc_in", [P, D], f32, kind="ExternalInput").ap()
    colidx_in = nc.dram_tensor("colidx_in", [P, P], f32, kind="ExternalInput").ap()
    ident_in = nc.dram_tensor("ident_in", [P, P], f32r, kind="ExternalInput").ap()
    ones_in = nc.dram_tensor("ones_in", [1, P], f32r, kind="ExternalInput").ap()
    idx_in = nc.dram_tensor("idx_in", [P, C], i32, kind="ExternalInput").ap()
    dloc_in = nc.dram_tensor("dloc_in", [P, C], f32, kind="ExternalInput").ap()
    s_out = nc.dram_tensor("s_out", [n_own, 1], f32, kind="ExternalOutput").ap()

    with tile.TileContext(nc) as tc:
        with tc.tile_pool(name="const", bufs=1) as const, \
             tc.tile_pool(name="dram", bufs=1, space="DRAM") as dram, \
             tc.tile_pool(name="zapool", bufs=1) as zapool, \
             tc.tile_pool(name="gpool", bufs=8) as gpool, \
             tc.tile_pool(name="opool", bufs=8) as opool, \
             tc.tile_pool(name="work", bufs=3) as work, \
             tc.tile_pool(name="apsum", bufs=2, space="PSUM") as apsum, \
             tc.tile_pool(name="zpsum", bufs=2, space="PSUM") as zpsum, \
             tc.tile_pool(name="tpsum", bufs=2, space="PSUM") as tpsum, \
             tc.tile_pool(name="ypsum", bufs=2, space="PSUM") as ypsum:

            # ------- resident constants
            colidx_sb = const.tile([P, P], f32)
            nc.sync.dma_start(out=colidx_sb[:], in_=colidx_in[:])
            ident_sb = const.tile([P, P], f32r)
            nc.sync.dma_start(out=ident_sb[:], in_=ident_in[:])
            ones_sb = const.tile([1, P], f32r)
            nc.sync.dma_start(out=ones_sb[:], in_=ones_in[:])
            wc_sb = const.tile([P, D], f32)
            nc.sync.dma_start(out=wc_sb[:], in_=wc_in[:])
            idx_sb = const.tile([P, C], i32)
            nc.sync.dma_start(out=idx_sb[:], in_=idx_in[:])
            dloc_sb = const.tile([P, C], f32)
            nc.sync.dma_start(out=dloc_sb[:], in_=dloc_in[:])
            w_sb = []
            b_sb = []
            for l in range(3):
                w_l = const.tile([P, KT * D], f32r, name=f"w_sb{l}")
                for k in range(KT):
                    nc.sync.dma_start(out=w_l[:, k * D:(k + 1) * D],
                                      in_=w_in[l][k * P:(k + 1) * P, :])
                w_sb.append(w_l)
                b_l = const.tile([1, D], f32r, name=f"b_sb{l}")
                nc.sync.dma_start(out=b_l[:], in_=b_in[l][:])
                b_sb.append(b_l)

            # region-A staging tiles (persist across a layer)
            zA = [zapool.tile([P, D], f32, name=f"zA{t}") for t in range(T)]

            # ------- inter-layer DRAM
            h_own_a = dram.tile([n_own, D], f32r)
            h_own_b = dram.tile([n_own, D], f32r)
            hfA_a = dram.tile([chg, D], f32r, addr_space="Shared")
            hfB_a = dram.tile([chg, D], f32r, addr_space="Shared")
            hfA_b = dram.tile([chg, D], f32r, addr_space="Shared")
            hfB_b = dram.tile([chg, D], f32r, addr_space="Shared")

            def gather_mm(h_src_ap, col, psum_ap, start, stop):
                g = gpool.tile([P, D], f32r, name="g")
                nc.gpsimd.indirect_dma_start(
                    out=g[:], out_offset=None, in_=h_src_ap,
                    in_offset=bass.IndirectOffsetOnAxis(
                        ap=idx_sb[:, col:col + 1], axis=0),
                )
                oh = opool.tile([P, P], f32r, name="oh")
                nc.vector.tensor_tensor(
                    out=oh[:],
                    in0=dloc_sb[:, col:col + 1].to_broadcast([P, P]),
                    in1=colidx_sb[:], op=mybir.AluOpType.is_equal)
                nc.tensor.matmul(out=psum_ap, lhsT=oh[:], rhs=g[:],
                                 start=start, stop=stop)

            def emit_phase_a(hA_ap):
                for t in range(T):
                    nA = int(K[t, 0])
                    if nA == 0:
                        nc.gpsimd.memset(zA[t][:], 0.0)
                        continue
                    psum_zA = apsum.tile([P, D], f32, space="PSUM",
                                         name="psum_zA")
                    for j in range(nA):
                        gather_mm(hA_ap, int(OA[t]) + j, psum_zA[:],
                                  j == 0, j == nA - 1)
                    nc.vector.tensor_copy(out=zA[t][:], in_=psum_zA[:])

            def emit_phase_b(l, hB_ap, h_own_ap, out_own_ap, ag_pair):
                for t in range(T):
                    nB = int(K[t, 1])
                    psum_z = zpsum.tile([P, D], f32, space="PSUM",
                                        name="psum_z")
                    for j in range(nB):
                        gather_mm(hB_ap, int(OB[t]) + j, psum_z[:],
                                  j == 0, False)
                    h_own_t = work.tile([P, D], f32r, name="h_own_t")
                    nc.sync.dma_start(out=h_own_t[:],
                                      in_=h_own_ap[t * P:(t + 1) * P, :])
                    nc.tensor.matmul(out=psum_z[:], lhsT=ident_sb[:],
                                     rhs=h_own_t[:], start=(nB == 0), stop=True)

                    z_sb = work.tile([P, D], f32r, name="z_sb")
                    nc.vector.tensor_tensor(out=z_sb[:], in0=psum_z[:],
                                            in1=zA[t][:],
                                            op=mybir.AluOpType.add)
                    zt_sb = work.tile([P, D], f32r, name="zt_sb")
                    for k in range(KT):
                        zt_ps = tpsum.tile([P, P], f32r, space="PSUM",
                                           name="zt_ps")
                        nc.tensor.transpose(out=zt_ps[:],
                                            in_=z_sb[:, k * P:(k + 1) * P],
                                            identity=ident_sb[:])
                        nc.vector.tensor_copy(out=zt_sb[:, k * P:(k + 1) * P],
                                              in_=zt_ps[:])

                    psum_y = ypsum.tile([P, D], f32, space="PSUM",
                                        name="psum_y")
                    for k in range(KT):
                        nc.tensor.matmul(out=psum_y[:],
                                         lhsT=zt_sb[:, k * P:(k + 1) * P],
                                         rhs=w_sb[l][:, k * D:(k + 1) * D],
                                         start=(k == 0), stop=False)
                    nc.tensor.matmul(out=psum_y[:], lhsT=ones_sb[:],
                                     rhs=b_sb[l][:], start=False, stop=True)

                    h_sb = work.tile([P, D], f32, name="h_sb")
                    nc.scalar.activation(out=h_sb[:], in_=psum_y[:],
                                         func=mybir.ActivationFunctionType.Relu)
                    if out_own_ap is not None:
                        nc.sync.dma_start(
                            out=out_own_ap[t * P:(t + 1) * P, :],
                            in_=h_sb[:].bitcast(f32r))
                        if t == T_A - 1:
                            _ag(out_own_ap, ag_pair[0], 0)
                        elif t == T - 1:
                            _ag(out_own_ap, ag_pair[1], 1)
                    else:
                        scratch = work.tile([P, D], f32, name="scratch")
                        nc.vector.tensor_tensor(out=scratch[:], in0=h_sb[:],
                                                in1=wc_sb[:],
                                                op=mybir.AluOpType.mult)
                        s_sb = work.tile([P, 1], f32, name="s_sb")
                        nc.vector.reduce_sum(out=s_sb[:], in_=scratch[:],
                                             axis=mybir.AxisListType.X)
                        nc.sync.dma_start(out=s_out[t * P:(t + 1) * P, :],
                                          in_=s_sb[:])

            def _ag(own_ap, hf_tile, half):
                nc.gpsimd.collective_compute(
                    "AllGather", mybir.AluOpType.bypass,
                    replica_groups=[list(range(NC))],
                    ins=[own_ap[half * ch:(half + 1) * ch, :].opt()],
                    outs=[hf_tile[:].opt()],
                )

            # layer 0: sources are the x inputs (replicated), no AG needed
            emit_phase_a(xA_in[:])
            emit_phase_b(0, xB_in[:], x_own[:], h_own_a[:], (hfA_a, hfB_a))
            # layer 1
            emit_phase_a(hfA_a[:])
            emit_phase_b(1, hfB_a[:], h_own_a[:], h_own_b[:], (hfA_b, hfB_b))
            # layer 2 (readout)
            emit_phase_a(hfA_b[:])
            emit_phase_b(2, hfB_b[:], h_own_b[:], None, None)

    nc.compile()
    return nc


def kernel(node_features, src, dst, graph_ids, num_graphs,
           W1, b1, W2, b2, W3, b3, Wc, bc):
    global LAST_EXEC_TIME_NS, LAST_RESULTS

    x = np.ascontiguousarray(np.asarray(node_features, dtype=np.float32))
    src = np.asarray(src).astype(np.int64)
    dst = np.asarray(dst).astype(np.int64)
    gids = np.asarray(graph_ids).astype(np.int64)
    G = int(np.asarray(num_graphs))
    W = [np.ascontiguousarray(np.asarray(w, np.float32)) for w in (W1, W2, W3)]
    b = [np.asarray(x_, np.float32).reshape(1, -1) for x_ in (b1, b2, b3)]
    wc = np.asarray(Wc, np.float32).reshape(-1)
    bc_v = np.asarray(bc, np.float32).reshape(-1)[0]

    N, D = x.shape
    q = math.ceil(N / NC)
    T = math.ceil(q / P)
    n_own = T * P
    if n_own % 2:
        n_own += P
        T = n_own // P
    N_pad = NC * n_own

    K, OA, OB, CA, CB, src_idx_T, dloc_T = _build_schedule(
        src, dst, N, q, T, n_own)

    sig = (N, D, CA, CB, n_own, tuple(int(k) for k in K.ravel()))
    if sig not in _program_cache:
        _program_cache[sig] = _build_program(D, T, K, OA, OB, CA, CB,
                                             n_own, N_pad)
    nc = _program_cache[sig]

    # padded per-core slabs; region-major split of the padded global layout
    x_own = np.zeros((NC, n_own, D), np.float32)
    for c in range(NC):
        lo, hi = c * q, min((c + 1) * q, N)
        x_own[c, :hi - lo] = x[lo:hi]
    ch = n_own // 2
    xA = np.ascontiguousarray(x_own[:, :ch].reshape(NC * ch, D))
    xB = np.ascontiguousarray(x_own[:, ch:].reshape(NC * ch, D))

    wc_rep = np.ascontiguousarray(np.tile(wc[None, :], (P, 1)).astype(np.float32))
    colidx = np.ascontiguousarray(np.tile(np.arange(P, dtype=np.float32), (P, 1)))
    ident = np.eye(P, dtype=np.float32)
    ones = np.ones((1, P), np.float32)

    in_maps = []
    for c in range(NC):
        in_maps.append({
            "xA_in": xA, "xB_in": xB,
            "x_own": np.ascontiguousarray(x_own[c]),
            "w0_in": W[0], "w1_in": W[1], "w2_in": W[2],
            "b0_in": b[0], "b1_in": b[1], "b2_in": b[2],
            "wc_in": wc_rep,
            "colidx_in": colidx,
            "ident_in": ident,
            "ones_in": ones,
            "idx_in": np.ascontiguousarray(src_idx_T[c]),
            "dloc_in": np.ascontiguousarray(dloc_T[c]),
        })

    r = bass_utils.run_bass_kernel_spmd(nc, in_maps,
                                        core_ids=list(range(NC)))
    LAST_EXEC_TIME_NS = r.exec_time_ns
    LAST_RESULTS = r

    parts = []
    for c in range(NC):
        lo, hi = c * q, min((c + 1) * q, N)
        parts.append(r.results[c]["s_out"][:hi - lo, 0])
    s = np.concatenate(parts)
    y = np.bincount(gids, weights=s.astype(np.float64), minlength=G)[:G]
    return (y.astype(np.float32) + bc_v)[:, None]


# revision 6
# speedup vs baseline: 1.3471x; 1.3471x over previous
"""GIN message-passing network (3 GINConv layers + per-graph sum readout) on
8 Trainium2 NeuronCores via Bass/Tile.

Sharding: nodes are partitioned contiguously across the 8 cores (graph
partitioning by destination); edges live on the core owning their dst node.
The padded global node layout is region-major over two halves (A = first half
of every core's slab, B = second half), so the inter-layer exchange is two
independent AllGathers (hfA, hfB), each with a single writer.

Per layer each core aggregates incoming-edge source features with
indirect-DMA row gathers reduced per 128-row dst tile via one-hot matmuls in
PSUM. Region-A contributions for all tiles are computed first and staged in
SBUF; region-B + self follow, which lets the next layer's region-A work (and
this layer's region-A AllGather) overlap the region-B AllGather. The dense
layer runs as z @ W + b with PE transposes feeding fp32r matmuls; ReLU on the
scalar engine. After layer 3 each core computes s = h3 @ Wc per node; the
host does the final per-graph segment sum (bincount over 50k scalars) + bc.
"""
import os
import sys
import math

sys.path.insert(0, "/opt/trn_rl_repo")

import numpy as np

import concourse.bass as bass
import concourse.bacc as bacc
import concourse.mybir as mybir
import concourse.tile as tile
from concourse import bass_utils

NC = 8
P = 128

f32 = mybir.dt.float32
f32r = mybir.dt.float32r
bf16 = mybir.dt.bfloat16
i32 = mybir.dt.int32

PAD_DLOC = 200.0  # one-hot no-match sentinel for padding slots

_program_cache = {}
LAST_EXEC_TIME_NS = None
LAST_RESULTS = None


def _build_schedule(src, dst, N, q, T, n_own):
    """Group edges by (dst core, 128-row dst tile, source region), chunk by 128.

    Global padded ids are region-major over two halves: slab rows [0, ch) of
    every core form region A (ids [0, ch*NC)), rows [ch, n_own) region B.
    Gather indices are stored LOCAL to their region's tensor.

    Returns (K, OA, OB, CA, CB, src_idx_T, dloc_T):
      K[t, r]    chunks for dst-tile t, region r (max over cores)
      OA/OB[t]   column offset of tile t's region-A/B chunks
      CA, CB     total chunks per region (columns: [A chunks | B chunks])
      src_idx_T  [NC, P, CA+CB] int32 region-local source ids (pad: 0)
      dloc_T     [NC, P, CA+CB] f32 dst index within tile (pad: PAD_DLOC)
    """
    E = src.shape[0]
    ch = n_own // 2
    c_e = dst // q
    dl = dst - c_e * q
    t_e = dl // P
    m_e = dl - t_e * P
    sc = src // q
    sr = src - sc * q            # slab row of the source on its owner core
    r_e = sr // ch               # source region 0/1
    srcl = (sr - r_e * ch) + sc * ch   # region-local id: core-major within region

    key = (c_e * T + t_e) * 2 + r_e
    order = np.argsort(key, kind="stable")
    key_s = key[order]
    srcl_s = srcl[order]
    m_s = m_e[order]

    counts = np.bincount(key, minlength=NC * T * 2)
    starts = np.concatenate([[0], np.cumsum(counts)])
    rank = np.arange(E, dtype=np.int64) - starts[key_s]

    K = np.ceil(counts.reshape(NC, T, 2).max(axis=0) / P).astype(np.int64)
    OA = np.concatenate([[0], np.cumsum(K[:, 0])])
    CA = int(OA[-1])
    OB = np.concatenate([[0], np.cumsum(K[:, 1])]) + CA
    CB = int(OB[-1]) - CA

    j = rank // P
    s = rank - j * P
    c_s = key_s // (T * 2)
    t_s = (key_s // 2) % T
    r_s = key_s % 2
    col = np.where(r_s == 0, OA[t_s], OB[t_s]) + j

    C = CA + CB
    src_idx_T = np.zeros((NC, P, C), np.int32)
    dloc_T = np.full((NC, P, C), PAD_DLOC, np.float32)
    src_idx_T[c_s, s, col] = srcl_s.astype(np.int32)
    dloc_T[c_s, s, col] = m_s.astype(np.float32)
    return K, OA, OB, CA, CB, src_idx_T, dloc_T


def _build_program(D, T, K, OA, OB, CA, CB, n_own, N_pad):
    KT = D // P
    C = CA + CB
    ch = n_own // 2
    chg = ch * NC  # rows per region tensor
    T_A = (ch + P - 1) // P  # tiles fully/partly in output chunk A (25 for T=49)

    nc = bacc.Bacc("TRN2", target_bir_lowering=False, debug=False,
                   num_devices=NC)

    xA_in = nc.dram_tensor("xA_in", [chg, D], bf16, kind="ExternalInput").ap()
    xB_in = nc.dram_tensor("xB_in", [chg, D], bf16, kind="ExternalInput").ap()
    x_own = nc.dram_tensor("x_own", [n_own, D], f32r, kind="ExternalInput").ap()
    w_in = [nc.dram_tensor(f"w{l}_in", [D, D], f32r, kind="ExternalInput").ap()
            for l in range(3)]
    b_in = [nc.dram_tensor(f"b{l}_in", [1, D], f32r, kind="ExternalInput").ap()
            for l in range(3)]
    wc_in = nc.dram_tensor("wc_in", [P, D], f32, kind="ExternalInput").ap()
    colidx_in = nc.dram_tensor("colidx_in", [P, P], f32, kind="ExternalInput").ap()
    ident_in = nc.dram_tensor("ident_in", [P, P], f32r, kind="ExternalInput").ap()
    ones_in = nc.dram_tensor("ones_in", [1, P], f32r, kind="ExternalInput").ap()
    idx_in = nc.dram_tensor("idx_in", [P, C], i32, kind="ExternalInput").ap()
    dloc_in = nc.dram_tensor("dloc_in", [P, C], f32, kind="ExternalInput").ap()
    s_out = nc.dram_tensor("s_out", [n_own, 1], f32, kind="ExternalOutput").ap()

    with tile.TileContext(nc) as tc:
        with tc.tile_pool(name="const", bufs=1) as const, \
             tc.tile_pool(name="dram", bufs=1, space="DRAM") as dram, \
             tc.tile_pool(name="zapool", bufs=1) as zapool, \
             tc.tile_pool(name="gpool", bufs=8) as gpool, \
             tc.tile_pool(name="opool", bufs=8) as opool, \
             tc.tile_pool(name="work", bufs=3) as work, \
             tc.tile_pool(name="apsum", bufs=2, space="PSUM") as apsum, \
             tc.tile_pool(name="zpsum", bufs=2, space="PSUM") as zpsum, \
             tc.tile_pool(name="tpsum", bufs=2, space="PSUM") as tpsum, \
             tc.tile_pool(name="ypsum", bufs=2, space="PSUM") as ypsum:

            # ------- resident constants
            colidx_sb = const.tile([P, P], f32)
            nc.sync.dma_start(out=colidx_sb[:], in_=colidx_in[:])
            ident_sb = const.tile([P, P], f32r)
            nc.sync.dma_start(out=ident_sb[:], in_=ident_in[:])
            ones_sb = const.tile([1, P], f32r)
            nc.sync.dma_start(out=ones_sb[:], in_=ones_in[:])
            wc_sb = const.tile([P, D], f32)
            nc.sync.dma_start(out=wc_sb[:], in_=wc_in[:])
            idx_sb = const.tile([P, C], i32)
            nc.sync.dma_start(out=idx_sb[:], in_=idx_in[:])
            dloc_sb = const.tile([P, C], f32)
            nc.sync.dma_start(out=dloc_sb[:], in_=dloc_in[:])
            w_sb = []
            b_sb = []
            for l in range(3):
                w_l = const.tile([P, KT * D], f32r, name=f"w_sb{l}")
                for k in range(KT):
                    nc.sync.dma_start(out=w_l[:, k * D:(k + 1) * D],
                                      in_=w_in[l][k * P:(k + 1) * P, :])
                w_sb.append(w_l)
                b_l = const.tile([1, D], f32r, name=f"b_sb{l}")
                nc.sync.dma_start(out=b_l[:], in_=b_in[l][:])
                b_sb.append(b_l)

            # region-A staging tiles (persist across a layer)
            zA = [zapool.tile([P, D], f32, name=f"zA{t}") for t in range(T)]

            # ------- inter-layer DRAM
            h_own_a = dram.tile([n_own, D], f32r)
            h_own_b = dram.tile([n_own, D], f32r)
            h_bf_a = dram.tile([n_own, D], bf16)
            h_bf_b = dram.tile([n_own, D], bf16)
            hfA_a = dram.tile([chg, D], bf16, addr_space="Shared")
            hfB_a = dram.tile([chg, D], bf16, addr_space="Shared")
            hfA_b = dram.tile([chg, D], bf16, addr_space="Shared")
            hfB_b = dram.tile([chg, D], bf16, addr_space="Shared")

            def gather_mm(h_src_ap, col, psum_ap, start, stop):
                g = gpool.tile([P, D], bf16, name="g")
                nc.gpsimd.indirect_dma_start(
                    out=g[:], out_offset=None, in_=h_src_ap,
                    in_offset=bass.IndirectOffsetOnAxis(
                        ap=idx_sb[:, col:col + 1], axis=0),
                )
                oh = opool.tile([P, P], bf16, name="oh")
                nc.vector.tensor_tensor(
                    out=oh[:],
                    in0=dloc_sb[:, col:col + 1].to_broadcast([P, P]),
                    in1=colidx_sb[:], op=mybir.AluOpType.is_equal)
                nc.tensor.matmul(out=psum_ap, lhsT=oh[:], rhs=g[:],
                                 start=start, stop=stop)

            def emit_phase_a(hA_ap):
                for t in range(T):
                    nA = int(K[t, 0])
                    if nA == 0:
                        nc.gpsimd.memset(zA[t][:], 0.0)
                        continue
                    psum_zA = apsum.tile([P, D], f32, space="PSUM",
                                         name="psum_zA")
                    for j in range(nA):
                        gather_mm(hA_ap, int(OA[t]) + j, psum_zA[:],
                                  j == 0, j == nA - 1)
                    nc.vector.tensor_copy(out=zA[t][:], in_=psum_zA[:])

            def emit_phase_b(l, hB_ap, h_own_ap, out_own_ap, out_bf_ap,
                             ag_pair):
                for t in range(T):
                    nB = int(K[t, 1])
                    psum_z = zpsum.tile([P, D], f32, space="PSUM",
                                        name="psum_z")
                    for j in range(nB):
                        gather_mm(hB_ap, int(OB[t]) + j, psum_z[:],
                                  j == 0, False)
                    h_own_t = work.tile([P, D], f32r, name="h_own_t")
                    nc.sync.dma_start(out=h_own_t[:],
                                      in_=h_own_ap[t * P:(t + 1) * P, :])
                    nc.tensor.matmul(out=psum_z[:], lhsT=ident_sb[:],
                                     rhs=h_own_t[:], start=(nB == 0), stop=True)

                    z_sb = work.tile([P, D], f32r, name="z_sb")
                    nc.vector.tensor_tensor(out=z_sb[:], in0=psum_z[:],
                                            in1=zA[t][:],
                                            op=mybir.AluOpType.add)
                    zt_sb = work.tile([P, D], f32r, name="zt_sb")
                    for k in range(KT):
                        zt_ps = tpsum.tile([P, P], f32r, space="PSUM",
                                           name="zt_ps")
                        nc.tensor.transpose(out=zt_ps[:],
                                            in_=z_sb[:, k * P:(k + 1) * P],
                                            identity=ident_sb[:])
                        nc.vector.tensor_copy(out=zt_sb[:, k * P:(k + 1) * P],
                                              in_=zt_ps[:])

                    psum_y = ypsum.tile([P, D], f32, space="PSUM",
                                        name="psum_y")
                    for k in range(KT):
                        nc.tensor.matmul(out=psum_y[:],
                                         lhsT=zt_sb[:, k * P:(k + 1) * P],
                                         rhs=w_sb[l][:, k * D:(k + 1) * D],
                                         start=(k == 0), stop=False)
                    nc.tensor.matmul(out=psum_y[:], lhsT=ones_sb[:],
                                     rhs=b_sb[l][:], start=False, stop=True)

                    h_sb = work.tile([P, D], f32, name="h_sb")
                    nc.scalar.activation(out=h_sb[:], in_=psum_y[:],
                                         func=mybir.ActivationFunctionType.Relu)
                    if out_own_ap is not None:
                        nc.sync.dma_start(
                            out=out_own_ap[t * P:(t + 1) * P, :],
                            in_=h_sb[:].bitcast(f32r))
                        h_bf_sb = work.tile([P, D], bf16, name="h_bf_sb")
                        nc.vector.tensor_copy(out=h_bf_sb[:], in_=h_sb[:])
                        nc.sync.dma_start(
                            out=out_bf_ap[t * P:(t + 1) * P, :],
                            in_=h_bf_sb[:])
                        if t == T_A - 1:
                            _ag(out_bf_ap, ag_pair[0], 0)
                        elif t == T - 1:
                            _ag(out_bf_ap, ag_pair[1], 1)
                    else:
                        scratch = work.tile([P, D], f32, name="scratch")
                        nc.vector.tensor_tensor(out=scratch[:], in0=h_sb[:],
                                                in1=wc_sb[:],
                                                op=mybir.AluOpType.mult)
                        s_sb = work.tile([P, 1], f32, name="s_sb")
                        nc.vector.reduce_sum(out=s_sb[:], in_=scratch[:],
                                             axis=mybir.AxisListType.X)
                        nc.sync.dma_start(out=s_out[t * P:(t + 1) * P, :],
                                          in_=s_sb[:])

            def _ag(own_ap, hf_tile, half):
                nc.gpsimd.collective_compute(
                    "AllGather", mybir.AluOpType.bypass,
                    replica_groups=[list(range(NC))],
                    ins=[own_ap[half * ch:(half + 1) * ch, :].opt()],
                    outs=[hf_tile[:].opt()],
                )

            # layer 0: sources are the x inputs (replicated), no AG needed
            emit_phase_a(xA_in[:])
            emit_phase_b(0, xB_in[:], x_own[:], h_own_a[:], h_bf_a[:],
                         (hfA_a, hfB_a))
            # layer 1
            emit_phase_a(hfA_a[:])
            emit_phase_b(1, hfB_a[:], h_own_a[:], h_own_b[:], h_bf_b[:],
                         (hfA_b, hfB_b))
            # layer 2 (readout)
            emit_phase_a(hfA_b[:])
            emit_phase_b(2, hfB_b[:], h_own_b[:], None, None, None)

    nc.compile()
    return nc


def kernel(node_features, src, dst, graph_ids, num_graphs,
           W1, b1, W2, b2, W3, b3, Wc, bc):
    global LAST_EXEC_TIME_NS, LAST_RESULTS

    x = np.ascontiguousarray(np.asarray(node_features, dtype=np.float32))
    src = np.asarray(src).astype(np.int64)
    dst = np.asarray(dst).astype(np.int64)
    gids = np.asarray(graph_ids).astype(np.int64)
    G = int(np.asarray(num_graphs))
    W = [np.ascontiguousarray(np.asarray(w, np.float32)) for w in (W1, W2, W3)]
    b = [np.asarray(x_, np.float32).reshape(1, -1) for x_ in (b1, b2, b3)]
    wc = np.asarray(Wc, np.float32).reshape(-1)
    bc_v = np.asarray(bc, np.float32).reshape(-1)[0]

    N, D = x.shape
    q = math.ceil(N / NC)
    T = math.ceil(q / P)
    n_own = T * P
    if n_own % 2:
        n_own += P
        T = n_own // P
    N_pad = NC * n_own

    K, OA, OB, CA, CB, src_idx_T, dloc_T = _build_schedule(
        src, dst, N, q, T, n_own)

    sig = (N, D, CA, CB, n_own, tuple(int(k) for k in K.ravel()))
    if sig not in _program_cache:
        _program_cache[sig] = _build_program(D, T, K, OA, OB, CA, CB,
                                             n_own, N_pad)
    nc = _program_cache[sig]

    # padded per-core slabs; region-major split of the padded global layout
    x_own = np.zeros((NC, n_own, D), np.float32)
    for c in range(NC):
        lo, hi = c * q, min((c + 1) * q, N)
        x_own[c, :hi - lo] = x[lo:hi]
    import ml_dtypes
    ch = n_own // 2
    xA = np.ascontiguousarray(
        x_own[:, :ch].reshape(NC * ch, D).astype(ml_dtypes.bfloat16))
    xB = np.ascontiguousarray(
        x_own[:, ch:].reshape(NC * ch, D).astype(ml_dtypes.bfloat16))

    wc_rep = np.ascontiguousarray(np.tile(wc[None, :], (P, 1)).astype(np.float32))
    colidx = np.ascontiguousarray(np.tile(np.arange(P, dtype=np.float32), (P, 1)))
    ident = np.eye(P, dtype=np.float32)
    ones = np.ones((1, P), np.float32)

    in_maps = []
    for c in range(NC):
        in_maps.append({
            "xA_in": xA, "xB_in": xB,
            "x_own": np.ascontiguousarray(x_own[c]),
            "w0_in": W[0], "w1_in": W[1], "w2_in": W[2],
            "b0_in": b[0], "b1_in": b[1], "b2_in": b[2],
            "wc_in": wc_rep,
            "colidx_in": colidx,
            "ident_in": ident,
            "ones_in": ones,
            "idx_in": np.ascontiguousarray(src_idx_T[c]),
            "dloc_in": np.ascontiguousarray(dloc_T[c]),
        })

    r = bass_utils.run_bass_kernel_spmd(nc, in_maps,
                                        core_ids=list(range(NC)))
    LAST_EXEC_TIME_NS = r.exec_time_ns
    LAST_RESULTS = r

    parts = []
    for c in range(NC):
        lo, hi = c * q, min((c + 1) * q, N)
        parts.append(r.results[c]["s_out"][:hi - lo, 0])
    s = np.concatenate(parts)
    y = np.bincount(gids, weights=s.astype(np.float64), minlength=G)[:G]
    return (y.astype(np.float32) + bc_v)[:, None]
